# revision 9
# baseline (speedup 1.0000x reference)
"""Trainium2 Bass kernel for nn_Attention_org_1554778161848.

Sharding: data-parallel over batch (B=8 -> 8 NeuronCores), weights replicated.

Per core pipeline (all matmuls bf16 with fp32 PSUM accumulation):
  phase 1: k/v 1x1 convs on PE; depthwise 3x3 as 9 'taps': 7 taps via PE
           diagonal-matmuls accumulated in PSUM (boundary handling via AP
           clipping + PSUM has_written semantics), the 2 aligned taps
           (dy=+-1, dx=0) via DVE scalar_tensor_tensor; l2norm(k) via ACT
           square+accum_out; k^T built with PE transposes -> DRAM; v -> DRAM.
  phase 2: q per branch in concat order [b4,b3,b2,b1] with channels permuted
           even/odd (grouped 3x3 conv becomes 2-band-diagonal); same stencil
           scheme; l2norm; transposes -> Q^T in DRAM.
  phase 3: attn = Q^T.T @ K^T streamed over 32 spatial tiles; per-branch
           InstanceNorm stats via ones-matmuls (literal 1/sqrt(960) scale and
           eps=1e-5 -- eps dominates variance, must match reference exactly);
           softmax = ACT Exp with per-partition scale/bias + accum_out row
           sums; wp is folded into probs ((wp@p)@v == wp@(p@v)); pv emits the
           final output.
"""
import json
import math

import numpy as np
import ml_dtypes

import concourse.bass as bass
import concourse.mybir as mybir
from concourse import masks
from concourse.tile import TileContext
from concourse.bass_utils import run_bass_kernel_spmd

BF16 = mybir.dt.bfloat16
F32 = mybir.dt.float32
AT = mybir.AluOpType
AF = mybir.ActivationFunctionType

CN = [64, 128, 256, 512]
KV = 960
HW = 4096
SCALE = 1.0 / math.sqrt(KV)
EPS_IN = 1e-5
BR_ORDER = [3, 2, 1, 0]
BR_C = [512, 256, 128, 64]
BR_OFF = [0, 512, 768, 896]
TAPS = [(dy, dx) for dy in (-1, 0, 1) for dx in (-1, 0, 1)]  # row-major, matches wk/wq reshape
DVE_TAPS = (1, 7)  # (-1,0), (+1,0): 64-element shifts, bf16-aligned for DVE
CENTER_TAP = 4     # (0,0): full coverage, must run first so start=True clears the bank
KV_TILES = [(0, 128), (128, 128), (256, 128), (384, 128),
            (512, 128), (640, 128), (768, 128), (896, 64)]
NB = 8
NT = 32
BR_MTILES = [[0, 1, 2, 3], [4, 5], [6], [7]]
BR_OF_M = [0, 0, 0, 0, 1, 1, 2, 3]


def _tiles_of(c):
    return [(0, 64)] if c == 64 else [(i * 128, 128) for i in range(c // 128)]


# ---------------------------------------------------------------- host prep

def perm_for(c):
    half = c // 2
    old = np.empty(c, np.int64)
    old[:half] = 2 * np.arange(half)
    old[half:] = 2 * np.arange(half) + 1
    return old


def _bf(x):
    return np.ascontiguousarray(np.asarray(x, np.float32)).astype(ml_dtypes.bfloat16)


def host_prep(ins):
    d = {}
    d["wmkT"] = _bf(np.asarray(ins["wmk"])[:, :, 0, 0].T)
    d["wmvT"] = _bf(np.asarray(ins["wmv"])[:, :, 0, 0].T)
    for nm, w in (("dwk", ins["wk"]), ("dwv", ins["wv"])):
        w9 = np.asarray(w)[:, 0].reshape(KV, 9).astype(np.float32)
        blk = np.zeros((KV, 9, 128), np.float32)
        for o0, p in KV_TILES:
            for i in range(p):
                blk[o0 + i, :, i] = w9[o0 + i]
        d[nm] = _bf(blk)
        d[nm + "f"] = np.ascontiguousarray(w9)  # fp32 per-channel tap weights
    for bi, c in enumerate(CN, 1):
        old = perm_for(c)
        half = c // 2
        wm = np.asarray(ins[f"wm{bi}"])[:, :, 0, 0]
        d[f"wmT{bi}"] = _bf(wm.T[:, old])
        wq = np.asarray(ins[f"wq{bi}"]).reshape(c, 2, 9).astype(np.float32)
        G = np.zeros((c, 9, c), np.float32)
        qwA = np.zeros((c, 9), np.float32)
        qwB = np.zeros((c, 9), np.float32)
        for j in range(c):
            o = old[j]
            g = o // 2
            G[g, :, j] = wq[o, 0, :]
            G[half + g, :, j] = wq[o, 1, :]
            qwA[j] = wq[o, 0, :]
            qwB[j] = wq[o, 1, :]
        d[f"qb{bi}"] = _bf(G)
        d[f"qwA{bi}"] = qwA
        d[f"qwB{bi}"] = qwB
        wp = np.asarray(ins[f"wp{bi}"])[:, :, 0, 0]
        d[f"wpT{bi}"] = _bf(wp.T[old, :])
    return d


def shard_inputs(ins, hp):
    emb_all = np.asarray(ins["emb_all"]).reshape(8, KV, HW)
    embs = [np.asarray(ins[f"emb{i}"]).reshape(8, CN[i - 1], HW) for i in range(1, 5)]
    maps = []
    for b in range(8):
        m = {"emb_all": _bf(emb_all[b])}
        for i in range(1, 5):
            m[f"e{i}"] = _bf(embs[i - 1][b])
        m.update(hp)
        maps.append(m)
    return maps


# ------------------------------------------------- walrus 1-wait workaround

def split_sync_waits(bir, limit=1):
    def fix_block(instrs):
        out = []
        for ins in instrs:
            si = ins.get("sync_info") or {}
            waits = si.get("on_wait") or []
            if len(waits) > limit:
                chunks = [waits[i:i + limit] for i in range(0, len(waits), limit)]
                for j, ch in enumerate(chunks[:-1]):
                    out.append({
                        "name": ins["name"] + f"-w{j}", "opcode": "Drain",
                        "engine": ins["engine"], "ins": [], "outs": [],
                        "is_reset_sema": False,
                        "sync_info": {"on_update": [], "on_wait": ch},
                        "debug": ins.get("debug"),
                    })
                ins["sync_info"]["on_wait"] = chunks[-1]
            out.append(ins)
        return out

    def walk(o):
        if isinstance(o, dict):
            for k, v in o.items():
                if k == "instructions" and isinstance(v, list):
                    o[k] = fix_block(v)
                else:
                    walk(v)
        elif isinstance(o, list):
            for v in o:
                walk(v)

    walk(bir)
    return bir


def patch_bass_serialization(nc):
    orig = nc.to_json_bytes
    nc.to_json_bytes = lambda: json.dumps(
        split_sync_waits(json.loads(orig()))).encode()
    return nc


# ----------------------------------------------------------- device builder

def _clip(nb, dy, dx):
    y0 = nb * 8
    oy0, oy1 = max(y0, -dy), min(y0 + 8, 64 - dy)
    ox0, ox1 = max(0, -dx), min(64, 64 - dx)
    return y0, oy0, oy1, ox0, ox1


def build_nc():
    nc = bass.Bass("TRN2", debug=False, num_devices=8)

    emb_all = nc.dram_tensor("emb_all", [KV, HW], BF16, kind="ExternalInput")
    e_in = {i: nc.dram_tensor(f"e{i}", [CN[i - 1], HW], BF16, kind="ExternalInput")
            for i in range(1, 5)}
    wmkT = nc.dram_tensor("wmkT", [KV, KV], BF16, kind="ExternalInput")
    wmvT = nc.dram_tensor("wmvT", [KV, KV], BF16, kind="ExternalInput")
    dwk = nc.dram_tensor("dwk", [KV, 9, 128], BF16, kind="ExternalInput")
    dwv = nc.dram_tensor("dwv", [KV, 9, 128], BF16, kind="ExternalInput")
    dwkf = nc.dram_tensor("dwkf", [KV, 9], F32, kind="ExternalInput")
    dwvf = nc.dram_tensor("dwvf", [KV, 9], F32, kind="ExternalInput")
    wmT, qb, qwA, qwB, wpT = {}, {}, {}, {}, {}
    for bi, c in enumerate(CN, 1):
        wmT[bi] = nc.dram_tensor(f"wmT{bi}", [c, c], BF16, kind="ExternalInput")
        qb[bi] = nc.dram_tensor(f"qb{bi}", [c, 9, c], BF16, kind="ExternalInput")
        qwA[bi] = nc.dram_tensor(f"qwA{bi}", [c, 9], F32, kind="ExternalInput")
        qwB[bi] = nc.dram_tensor(f"qwB{bi}", [c, 9], F32, kind="ExternalInput")
        wpT[bi] = nc.dram_tensor(f"wpT{bi}", [c, c], BF16, kind="ExternalInput")

    v_sp = nc.dram_tensor("v_sp", [KV, HW], BF16)
    kT_sp = nc.dram_tensor("kT_sp", [NT, 128, KV], BF16)
    qT_sp = nc.dram_tensor("qT_sp", [NT, 128, KV], BF16)
    out_cat = nc.dram_tensor("out_cat", [KV, HW], F32, kind="ExternalOutput")

    with TileContext(nc) as tc:
        with tc.tile_pool(name="persist", bufs=1) as persist:
            ident = persist.tile([128, 128], BF16, tag="ident")
            masks.make_identity(nc, ident[:])
            ones_bf = persist.tile([128, 1], BF16, tag="ones_bf")
            nc.vector.memset(ones_bf[:], 1.0)

            # ====================== phase 1: k and v ======================
            with tc.tile_pool(name="ph1emb", bufs=1) as ph1emb:
                emb_t = []
                for ti, (o0, p) in enumerate(KV_TILES):
                    t = ph1emb.tile([p, HW], BF16, tag=f"emba{ti}")
                    nc.sync.dma_start(t[:], emb_all[o0:o0 + p, :])
                    emb_t.append(t)

                for which, (wT_d, dw_d, dwf_d) in enumerate(
                        ((wmkT, dwk, dwkf), (wmvT, dwv, dwvf))):
                    is_k = which == 0
                    with (
                        tc.tile_pool(name=f"ph1w{which}", bufs=1) as phw,
                        tc.tile_pool(name=f"ph1_{which}", bufs=2) as ph,
                    ):
                        wt = []
                        for ti, (o0, p) in enumerate(KV_TILES):
                            t = phw.tile([p, KV], BF16, tag=f"w1x1_{ti}")
                            nc.sync.dma_start(t[:], wT_d[o0:o0 + p, :])
                            wt.append(t)

                        for mi, (m0, mp) in enumerate(KV_TILES):
                            xpre = ph.tile([mp, HW], BF16, tag="xpre")
                            with tc.tile_pool(name="ps1", bufs=4,
                                              space="PSUM") as ps1:
                                for nb in range(NB):
                                    pt = ps1.tile([mp, 512], F32, tag="pre")
                                    for kt, (k0, kp) in enumerate(KV_TILES):
                                        nc.tensor.matmul(
                                            pt[:], wt[kt][:, m0:m0 + mp],
                                            emb_t[kt][:, nb * 512:(nb + 1) * 512],
                                            start=(kt == 0), stop=(kt == 7))
                                    nc.scalar.copy(
                                        xpre[:, nb * 512:(nb + 1) * 512], pt[:])

                            dwt = ph.tile([mp, 9, 128], BF16, tag="dwt", bufs=1)
                            nc.sync.dma_start(dwt[:], dw_d[m0:m0 + mp, :, :])
                            dwf = ph.tile([mp, 9], F32, tag="dwf")
                            nc.sync.dma_start(dwf[:], dwf_d[m0:m0 + mp, :])

                            x3 = xpre[:].rearrange("c (y x) -> c y x", x=64)
                            part = ph.tile([mp, HW], BF16, tag="part", bufs=1)
                            nc.vector.memset(part[:, 0:64], 0.0)
                            nc.vector.tensor_scalar(
                                part[:, 64:HW], xpre[:, 0:HW - 64],
                                dwf[:, 1:2], None, AT.mult)
                            nc.vector.scalar_tensor_tensor(
                                part[:, 0:HW - 64], xpre[:, 64:HW], dwf[:, 7:8],
                                part[:, 0:HW - 64], AT.mult, AT.add)

                            xdw = ph.tile([mp, HW], BF16, tag="xdw")
                            pe_taps = [CENTER_TAP] + [
                                t for t in range(9)
                                if t != CENTER_TAP and t not in DVE_TAPS]
                            with tc.tile_pool(name="ps2", bufs=4,
                                              space="PSUM") as ps2:
                                for nb in range(NB):
                                    pt = ps2.tile([mp, 512], F32, tag="dwp")
                                    pt3 = pt[:].rearrange("c (y x) -> c y x", x=64)
                                    for j, t in enumerate(pe_taps):
                                        dy, dx = TAPS[t]
                                        y0, oy0, oy1, ox0, ox1 = _clip(nb, dy, dx)
                                        if oy1 <= oy0:
                                            continue
                                        nc.tensor.matmul(
                                            pt3[:, oy0 - y0:oy1 - y0, ox0:ox1],
                                            dwt[:, t, 0:mp],
                                            x3[:, oy0 + dy:oy1 + dy,
                                               ox0 + dx:ox1 + dx],
                                            start=(j == 0),
                                            stop=(j == len(pe_taps) - 1))
                                    nc.vector.scalar_tensor_tensor(
                                        xdw[:, nb * 512:(nb + 1) * 512], pt[:],
                                        1.0, part[:, nb * 512:(nb + 1) * 512],
                                        AT.mult, AT.add)

                            if is_k:
                                sq = ph.tile([mp, HW], BF16, tag="sqs", bufs=1)
                                ss = ph.tile([mp, 1], F32, tag="ss")
                                nc.scalar.activation(sq[:], xdw[:], AF.Square,
                                                     accum_out=ss[:])
                                rt = ph.tile([mp, 1], F32, tag="rt")
                                nc.scalar.sqrt(rt[:], ss[:])
                                rs = ph.tile([mp, 1], F32, tag="rs")
                                nc.vector.reciprocal(rs[:], rt[:])
                                xs = ph.tile([mp, HW], BF16, tag="xs")
                                nc.vector.tensor_scalar(xs[:], xdw[:], rs[:],
                                                        None, AT.mult)
                                with tc.tile_pool(name="pst", bufs=4,
                                                  space="PSUM") as pst:
                                    for cb in range(NT):
                                        ptr = pst.tile([128, mp], BF16, tag="tr")
                                        nc.tensor.transpose(
                                            ptr[:, 0:mp],
                                            xs[:, cb * 128:(cb + 1) * 128],
                                            ident[0:mp, 0:mp])
                                        st = ph.tile([128, mp], BF16, tag="trs")
                                        nc.scalar.copy(st[:], ptr[:, 0:mp])
                                        nc.sync.dma_start(
                                            kT_sp[cb, :, m0:m0 + mp], st[:])
                            else:
                                nc.sync.dma_start(v_sp[m0:m0 + mp, :], xdw[:])

            # ====================== phase 2: q branches ======================
            for bri, bi in enumerate([4, 3, 2, 1]):
                c = CN[bi - 1]
                half = c // 2
                br_off = BR_OFF[bri]
                tiles = _tiles_of(c)
                nti = len(tiles)
                with tc.tile_pool(name=f"ph2_{bi}", bufs=2) as ph:
                    wmt, et = [], []
                    for kt, (k0, kp) in enumerate(tiles):
                        t = ph.tile([kp, c], BF16, tag=f"wm_{kt}", bufs=1)
                        nc.sync.dma_start(t[:], wmT[bi][k0:k0 + kp, :])
                        wmt.append(t)
                        t2 = ph.tile([kp, HW], BF16, tag=f"e_{kt}", bufs=1)
                        nc.sync.dma_start(t2[:], e_in[bi][k0:k0 + kp, :])
                        et.append(t2)

                    qpre = []
                    for mi, (m0, mp) in enumerate(tiles):
                        xp = ph.tile([mp, HW], BF16, tag=f"qpre_{mi}", bufs=1)
                        with tc.tile_pool(name="ps3", bufs=4, space="PSUM") as ps3:
                            for nb in range(NB):
                                pt = ps3.tile([mp, 512], F32, tag="qp")
                                for kt, (k0, kp) in enumerate(tiles):
                                    nc.tensor.matmul(
                                        pt[:], wmt[kt][:, m0:m0 + mp],
                                        et[kt][:, nb * 512:(nb + 1) * 512],
                                        start=(kt == 0), stop=(kt == nti - 1))
                                nc.scalar.copy(xp[:, nb * 512:(nb + 1) * 512],
                                               pt[:])
                        qpre.append(xp)

                    for mi, (m0, mp) in enumerate(tiles):
                        if c >= 256:
                            a_ti = mi if m0 + mp <= half else mi - nti // 2
                            b_ti = a_ti + nti // 2
                            band_tiles = [a_ti, b_ti]
                            use_dve = True
                        else:
                            band_tiles = [0]
                            use_dve = False

                        part = None
                        if use_dve:
                            qa = ph.tile([mp, 9], F32, tag="qa")
                            qbv = ph.tile([mp, 9], F32, tag="qbv")
                            nc.sync.dma_start(qa[:], qwA[bi][m0:m0 + mp, :])
                            nc.sync.dma_start(qbv[:], qwB[bi][m0:m0 + mp, :])
                            srcA, srcB = qpre[band_tiles[0]], qpre[band_tiles[1]]
                            part = ph.tile([mp, HW], BF16, tag="qpart", bufs=1)
                            nc.vector.memset(part[:, 0:64], 0.0)
                            nc.vector.tensor_scalar(
                                part[:, 64:HW], srcA[:, 0:HW - 64], qa[:, 1:2],
                                None, AT.mult)
                            nc.vector.scalar_tensor_tensor(
                                part[:, 64:HW], srcB[:, 0:HW - 64], qbv[:, 1:2],
                                part[:, 64:HW], AT.mult, AT.add)
                            nc.vector.scalar_tensor_tensor(
                                part[:, 0:HW - 64], srcA[:, 64:HW], qa[:, 7:8],
                                part[:, 0:HW - 64], AT.mult, AT.add)
                            nc.vector.scalar_tensor_tensor(
                                part[:, 0:HW - 64], srcB[:, 64:HW], qbv[:, 7:8],
                                part[:, 0:HW - 64], AT.mult, AT.add)
                            pe_taps = [CENTER_TAP] + [
                                t for t in range(9)
                                if t != CENTER_TAP and t not in DVE_TAPS]
                        else:
                            pe_taps = [CENTER_TAP] + [
                                t for t in range(9) if t != CENTER_TAP]

                        qdw = ph.tile([mp, HW], BF16, tag="qdw")
                        bmats = []
                        for z, ti_src in enumerate(band_tiles):
                            k0b, kpb = tiles[ti_src]
                            bm = ph.tile([kpb, 9, mp], BF16, tag=f"qbm_{z}")
                            nc.sync.dma_start(
                                bm[:], qb[bi][k0b:k0b + kpb, :, m0:m0 + mp])
                            bmats.append((bm, qpre[ti_src]))
                        n_mm = len(pe_taps) * len(bmats)
                        with tc.tile_pool(name="ps4", bufs=4, space="PSUM") as ps4:
                            for nb in range(NB):
                                pt = ps4.tile([mp, 512], F32, tag="qdwp")
                                pt3 = pt[:].rearrange("c (y x) -> c y x", x=64)
                                j = 0
                                for t in pe_taps:
                                    dy, dx = TAPS[t]
                                    y0, oy0, oy1, ox0, ox1 = _clip(nb, dy, dx)
                                    for (bm, src) in bmats:
                                        if oy1 <= oy0:
                                            j += 1
                                            continue
                                        s3 = src[:].rearrange(
                                            "c (y x) -> c y x", x=64)
                                        nc.tensor.matmul(
                                            pt3[:, oy0 - y0:oy1 - y0, ox0:ox1],
                                            bm[:, t, :],
                                            s3[:, oy0 + dy:oy1 + dy,
                                               ox0 + dx:ox1 + dx],
                                            start=(j == 0), stop=(j == n_mm - 1))
                                        j += 1
                                if use_dve:
                                    nc.vector.scalar_tensor_tensor(
                                        qdw[:, nb * 512:(nb + 1) * 512], pt[:],
                                        1.0, part[:, nb * 512:(nb + 1) * 512],
                                        AT.mult, AT.add)
                                else:
                                    nc.scalar.copy(
                                        qdw[:, nb * 512:(nb + 1) * 512], pt[:])

                        sq = ph.tile([mp, HW], BF16, tag="qsq", bufs=1)
                        ss = ph.tile([mp, 1], F32, tag="qss")
                        nc.scalar.activation(sq[:], qdw[:], AF.Square,
                                             accum_out=ss[:])
                        rt = ph.tile([mp, 1], F32, tag="qrt")
                        nc.scalar.sqrt(rt[:], ss[:])
                        rs = ph.tile([mp, 1], F32, tag="qrs")
                        nc.vector.reciprocal(rs[:], rt[:])
                        xs = ph.tile([mp, HW], BF16, tag="qxs")
                        nc.vector.tensor_scalar(xs[:], qdw[:], rs[:], None, AT.mult)
                        with tc.tile_pool(name="pst2", bufs=4, space="PSUM") as pst:
                            for cb in range(NT):
                                ptr = pst.tile([128, mp], BF16, tag="qtr")
                                nc.tensor.transpose(
                                    ptr[:, 0:mp], xs[:, cb * 128:(cb + 1) * 128],
                                    ident[0:mp, 0:mp])
                                st = ph.tile([128, mp], BF16, tag="qtrs")
                                nc.scalar.copy(st[:], ptr[:, 0:mp])
                                nc.sync.dma_start(
                                    qT_sp[cb, :, br_off + m0:br_off + m0 + mp],
                                    st[:])

            # ====================== phase 3 ======================
            attn_bf = []
            for mi, (m0, mp) in enumerate(KV_TILES):
                attn_bf.append(persist.tile([mp, KV], BF16, tag=f"attn_{mi}", name=f"attn_{mi}"))

            with tc.tile_pool(name="ph3", bufs=3) as ph3:
                for half_i in range(2):
                    ms = list(range(4 * half_i, 4 * half_i + 4))
                    with tc.tile_pool(name="psA", bufs=1, space="PSUM") as psA:
                        pts = {}
                        for mi in ms:
                            m0, mp = KV_TILES[mi]
                            pts[mi] = psA.tile([mp, KV], F32, tag=f"attnp{mi % 4}", name=f"attnp{mi}")
                        for n in range(NT):
                            kTt = ph3.tile([128, KV], BF16, tag="kTt")
                            qTt = ph3.tile([128, KV], BF16, tag="qTt")
                            nc.sync.dma_start(kTt[:], kT_sp[n, :, :])
                            nc.sync.dma_start(qTt[:], qT_sp[n, :, :])
                            for mi in ms:
                                m0, mp = KV_TILES[mi]
                                for s0, sw in ((0, 512), (512, 448)):
                                    nc.tensor.matmul(
                                        pts[mi][:, s0:s0 + sw],
                                        qTt[:, m0:m0 + mp],
                                        kTt[:, s0:s0 + sw],
                                        start=(n == 0), stop=(n == NT - 1))
                        for mi in ms:
                            nc.scalar.copy(attn_bf[mi][:], pts[mi][:])

                # per-branch stats
                stat_sc = []
                with (
                    tc.tile_pool(name="psS", bufs=1, space="PSUM") as psS,
                    tc.tile_pool(name="psB", bufs=1, space="PSUM") as psB,
                ):
                    for bri in range(4):
                        mts = BR_MTILES[bri]
                        c = BR_C[bri]
                        s1p = psS.tile([1, KV], F32, tag="s1")
                        s2p = psS.tile([1, KV], F32, tag="s2")
                        for j, mi in enumerate(mts):
                            m0, mp = KV_TILES[mi]
                            sqt = ph3.tile([mp, KV], BF16, tag="asq")
                            nc.scalar.activation(sqt[:], attn_bf[mi][:], AF.Square)
                            for s0, sw in ((0, 512), (512, 448)):
                                nc.tensor.matmul(
                                    s1p[:, s0:s0 + sw], ones_bf[0:mp, :],
                                    attn_bf[mi][:, s0:s0 + sw],
                                    start=(j == 0), stop=(j == len(mts) - 1))
                                nc.tensor.matmul(
                                    s2p[:, s0:s0 + sw], ones_bf[0:mp, :],
                                    sqt[:, s0:s0 + sw],
                                    start=(j == 0), stop=(j == len(mts) - 1))
                        s1r = ph3.tile([1, KV], F32, tag="s1r")
                        s2r = ph3.tile([1, KV], F32, tag="s2r")
                        nc.scalar.copy(s1r[:], s1p[:])
                        nc.scalar.copy(s2r[:], s2p[:])
                        s1s = ph3.tile([1, 1], F32, tag="s1s")
                        s2s = ph3.tile([1, 1], F32, tag="s2s")
                        nc.vector.tensor_reduce(s1s[:], s1r[:], mybir.AxisListType.X, AT.add)
                        nc.vector.tensor_reduce(s2s[:], s2r[:], mybir.AxisListType.X, AT.add)
                        n_el = float(c * KV)
                        mu = ph3.tile([1, 1], F32, tag="mu")
                        nc.vector.tensor_scalar(mu[:], s1s[:], 1.0 / n_el, None,
                                                AT.mult)
                        ex2 = ph3.tile([1, 1], F32, tag="ex2")
                        nc.vector.tensor_scalar(ex2[:], s2s[:], 1.0 / n_el, None,
                                                AT.mult)
                        mu2 = ph3.tile([1, 1], F32, tag="mu2")
                        nc.vector.tensor_tensor(mu2[:], mu[:], mu[:], AT.mult)
                        var = ph3.tile([1, 1], F32, tag="var")
                        nc.vector.tensor_tensor(var[:], ex2[:], mu2[:], AT.subtract)
                        vs = ph3.tile([1, 1], F32, tag="vs")
                        nc.vector.tensor_scalar(vs[:], var[:], SCALE * SCALE,
                                                EPS_IN, AT.mult, AT.add)
                        sd = ph3.tile([1, 1], F32, tag="sd")
                        nc.scalar.sqrt(sd[:], vs[:])
                        rsb = ph3.tile([1, 1], F32, tag="rsb")
                        nc.vector.reciprocal(rsb[:], sd[:])
                        scl = ph3.tile([1, 1], F32, tag="scl")
                        nc.vector.tensor_scalar(scl[:], rsb[:], SCALE, None, AT.mult)
                        bia = ph3.tile([1, 1], F32, tag="bia")
                        nc.vector.tensor_tensor(bia[:], mu[:], scl[:], AT.mult)
                        nc.vector.tensor_scalar(bia[:], bia[:], -1.0, None, AT.mult)
                        onesr_f = ph3.tile([1, 128], F32, tag="onesr_f")
                        nc.vector.memset(onesr_f[:], 1.0)
                        s_ps = psB.tile([128, 1], F32, tag="bps1")
                        b_ps = psB.tile([128, 1], F32, tag="bps2")
                        nc.tensor.matmul(s_ps[:], onesr_f[:], scl[:],
                                         start=True, stop=True)
                        nc.tensor.matmul(b_ps[:], onesr_f[:], bia[:],
                                         start=True, stop=True)
                        sclv = ph3.tile([128, 1], F32, tag=f"sclv_{bri}")
                        biav = ph3.tile([128, 1], F32, tag=f"biav_{bri}")
                        nc.scalar.copy(sclv[:], s_ps[:])
                        nc.scalar.copy(biav[:], b_ps[:])
                        stat_sc.append((sclv, biav))

                # softmax
                probs_bf = []
                for mi, (m0, mp) in enumerate(KV_TILES):
                    sclv, biav = stat_sc[BR_OF_M[mi]]
                    pb = persist.tile([mp, KV], BF16, tag=f"probs_{mi}")
                    rsum = ph3.tile([mp, 1], F32, tag="rsum")
                    nc.scalar.activation(
                        pb[:], attn_bf[mi][:], AF.Exp,
                        bias=biav[0:mp, :], scale=sclv[0:mp, :],
                        accum_out=rsum[:])
                    rinv = ph3.tile([mp, 1], F32, tag="rinv")
                    nc.vector.reciprocal(rinv[:], rsum[:])
                    nc.vector.tensor_scalar(pb[:], pb[:], rinv[:], None, AT.mult)
                    probs_bf.append(pb)

                # fold wp into probs: pw[dt][d, co_global]
                pw_bf = []
                for dt, (d0, dp) in enumerate(KV_TILES):
                    pw_bf.append(persist.tile([dp, KV], BF16, tag=f"pw_{dt}", name=f"pw_{dt}"))
                with tc.tile_pool(name="psF", bufs=1, space="PSUM") as psF:
                    wpt = {}
                    for bri, bi in enumerate([4, 3, 2, 1]):
                        ts = []
                        for kt, (k0, kp) in enumerate(_tiles_of(BR_C[bri])):
                            t = ph3.tile([kp, BR_C[bri]], BF16,
                                         tag=f"wp_{bri}_{kt}")
                            nc.sync.dma_start(t[:], wpT[bi][k0:k0 + kp, :])
                            ts.append(t)
                        wpt[bri] = ts
                    for dt, (d0, dp) in enumerate(KV_TILES):
                        pf = psF.tile([dp, KV], F32, tag=f"pf{dt % 4}")
                        for bri in range(4):
                            c, off = BR_C[bri], BR_OFF[bri]
                            tl = _tiles_of(c)
                            for kt, (k0, kp) in enumerate(tl):
                                mi = (off + k0) // 128
                                nc.tensor.matmul(
                                    pf[:, off:off + c],
                                    probs_bf[mi][:, d0:d0 + dp],
                                    wpt[bri][kt][:],
                                    # bank 1 = cols 0:512 (b4); bank 2 =
                                    # cols 512:960 (b3 clears, b2/b1 land on
                                    # unwritten cells -> overwrite)
                                    start=(bri <= 1 and kt == 0),
                                    stop=(kt == len(tl) - 1))
                        nc.scalar.copy(pw_bf[dt][:], pf[:])

                # pv -> final output
                with tc.tile_pool(name="psO", bufs=1, space="PSUM") as psO:
                    for nb in range(NB):
                        vts = []
                        for dt, (d0, dp) in enumerate(KV_TILES):
                            vt = ph3.tile([dp, 512], BF16, tag=f"vt{dt}")
                            nc.sync.dma_start(
                                vt[:], v_sp[d0:d0 + dp, nb * 512:(nb + 1) * 512])
                            vts.append(vt)
                        for mo, (m0, mp) in enumerate(KV_TILES):
                            po = psO.tile([mp, 512], F32, tag=f"po{mo % 4}")
                            for dt in range(8):
                                nc.tensor.matmul(
                                    po[:], pw_bf[dt][:, m0:m0 + mp], vts[dt][:],
                                    start=(dt == 0), stop=(dt == 7))
                            ot = ph3.tile([mp, 512], F32, tag="ot")
                            nc.scalar.copy(ot[:], po[:])
                            nc.sync.dma_start(
                                out_cat[m0:m0 + mp, nb * 512:(nb + 1) * 512],
                                ot[:])

    return nc


# ---------------------------------------------------------------- entry

_CACHE = {}


def _get_nc():
    if "nc" not in _CACHE:
        _CACHE["nc"] = patch_bass_serialization(build_nc())
    return _CACHE["nc"]


def kernel(**inputs):
    ins = {k: np.asarray(v) for k, v in inputs.items()}
    hp = host_prep(ins)
    in_maps = shard_inputs(ins, hp)
    nc = _get_nc()
    res = run_bass_kernel_spmd(nc, in_maps, core_ids=list(range(8)))
    outs = []
    for i in range(4):
        c = CN[i]
        bri = BR_ORDER.index(i)
        r0 = BR_OFF[bri]
        arr = np.stack([
            res.results[b]["out_cat"][r0:r0 + c].reshape(c, 64, 64)
            for b in range(8)
        ])
        outs.append(arr.astype(np.float32))
    return tuple(outs)


# revision 19
# speedup vs baseline: 1.4213x; 1.4213x over previous
"""Trainium2 Bass kernel for nn_Attention_org_1554778161848.

Sharding: data-parallel over batch (B=8 -> 8 NeuronCores), weights replicated.

Per core pipeline (all matmuls bf16 with fp32 PSUM accumulation):
  phase 1: k/v 1x1 convs on PE; depthwise 3x3 as 9 'taps': 7 taps via PE
           diagonal-matmuls accumulated in PSUM (boundary handling via AP
           clipping + PSUM has_written semantics), the 2 aligned taps
           (dy=+-1, dx=0) via DVE scalar_tensor_tensor; l2norm(k) via ACT
           square+accum_out; k^T built with PE transposes -> DRAM; v -> DRAM.
  phase 2: q per branch in concat order [b4,b3,b2,b1] with channels permuted
           even/odd (grouped 3x3 conv becomes 2-band-diagonal); same stencil
           scheme; l2norm; transposes -> Q^T in DRAM.
  phase 3: attn = Q^T.T @ K^T streamed over 32 spatial tiles; per-branch
           InstanceNorm stats via ones-matmuls (literal 1/sqrt(960) scale and
           eps=1e-5 -- eps dominates variance, must match reference exactly);
           softmax = ACT Exp with per-partition scale/bias + accum_out row
           sums; wp is folded into probs ((wp@p)@v == wp@(p@v)); pv emits the
           final output.
"""
import json
import math

import numpy as np
import ml_dtypes

import concourse.bass as bass
import concourse.mybir as mybir
from concourse import masks
from concourse.tile import TileContext
from concourse.bass_utils import run_bass_kernel_spmd

BF16 = mybir.dt.bfloat16
F32 = mybir.dt.float32
AT = mybir.AluOpType
AF = mybir.ActivationFunctionType

CN = [64, 128, 256, 512]
KV = 960
HW = 4096
SCALE = 1.0 / math.sqrt(KV)
EPS_IN = 1e-5
BR_ORDER = [3, 2, 1, 0]
BR_C = [512, 256, 128, 64]
BR_OFF = [0, 512, 768, 896]
TAPS = [(dy, dx) for dy in (-1, 0, 1) for dx in (-1, 0, 1)]  # row-major, matches wk/wq reshape
DVE_TAPS = (1, 7)  # (-1,0), (+1,0): 64-element shifts, bf16-aligned for DVE
CENTER_TAP = 4     # (0,0): full coverage, must run first so start=True clears the bank
KV_TILES = [(0, 128), (128, 128), (256, 128), (384, 128),
            (512, 128), (640, 128), (768, 128), (896, 64)]
NB = 8
NT = 32
BR_MTILES = [[0, 1, 2, 3], [4, 5], [6], [7]]
BR_OF_M = [0, 0, 0, 0, 1, 1, 2, 3]


def _tiles_of(c):
    return [(0, 64)] if c == 64 else [(i * 128, 128) for i in range(c // 128)]


# ---------------------------------------------------------------- host prep

def perm_for(c):
    half = c // 2
    old = np.empty(c, np.int64)
    old[:half] = 2 * np.arange(half)
    old[half:] = 2 * np.arange(half) + 1
    return old


def _bf(x):
    return np.ascontiguousarray(np.asarray(x, np.float32)).astype(ml_dtypes.bfloat16)


def host_prep(ins):
    d = {}
    d["wmkT"] = _bf(np.asarray(ins["wmk"])[:, :, 0, 0].T)
    d["wmvT"] = _bf(np.asarray(ins["wmv"])[:, :, 0, 0].T)
    for nm, w in (("dwk", ins["wk"]), ("dwv", ins["wv"])):
        w9 = np.asarray(w)[:, 0].reshape(KV, 9).astype(np.float32)
        blk = np.zeros((KV, 9, 128), np.float32)
        for o0, p in KV_TILES:
            for i in range(p):
                blk[o0 + i, :, i] = w9[o0 + i]
        d[nm] = _bf(blk)
        d[nm + "f"] = np.ascontiguousarray(w9)  # fp32 per-channel tap weights
    for bi, c in enumerate(CN, 1):
        old = perm_for(c)
        half = c // 2
        wm = np.asarray(ins[f"wm{bi}"])[:, :, 0, 0]
        d[f"wmT{bi}"] = _bf(wm.T[:, old])
        wq = np.asarray(ins[f"wq{bi}"]).reshape(c, 2, 9).astype(np.float32)
        G = np.zeros((c, 9, c), np.float32)
        qwA = np.zeros((c, 9), np.float32)
        qwB = np.zeros((c, 9), np.float32)
        for j in range(c):
            o = old[j]
            g = o // 2
            G[g, :, j] = wq[o, 0, :]
            G[half + g, :, j] = wq[o, 1, :]
            qwA[j] = wq[o, 0, :]
            qwB[j] = wq[o, 1, :]
        d[f"qb{bi}"] = _bf(G)
        d[f"qwA{bi}"] = qwA
        d[f"qwB{bi}"] = qwB
        wp = np.asarray(ins[f"wp{bi}"])[:, :, 0, 0]
        d[f"wpT{bi}"] = _bf(wp.T[old, :])
    return d


def shard_inputs(ins, hp):
    emb_all = np.asarray(ins["emb_all"]).reshape(8, KV, HW)
    embs = [np.asarray(ins[f"emb{i}"]).reshape(8, CN[i - 1], HW) for i in range(1, 5)]
    maps = []
    for b in range(8):
        m = {"emb_all": _bf(emb_all[b])}
        for i in range(1, 5):
            m[f"e{i}"] = _bf(embs[i - 1][b])
        m.update(hp)
        maps.append(m)
    return maps


# ------------------------------------------------- walrus 1-wait workaround

def split_sync_waits(bir, limit=1):
    def fix_block(instrs):
        out = []
        for ins in instrs:
            si = ins.get("sync_info") or {}
            waits = si.get("on_wait") or []
            if len(waits) > limit:
                chunks = [waits[i:i + limit] for i in range(0, len(waits), limit)]
                for j, ch in enumerate(chunks[:-1]):
                    out.append({
                        "name": ins["name"] + f"-w{j}", "opcode": "Drain",
                        "engine": ins["engine"], "ins": [], "outs": [],
                        "is_reset_sema": False,
                        "sync_info": {"on_update": [], "on_wait": ch},
                        "debug": ins.get("debug"),
                    })
                ins["sync_info"]["on_wait"] = chunks[-1]
            out.append(ins)
        return out

    def walk(o):
        if isinstance(o, dict):
            for k, v in o.items():
                if k == "instructions" and isinstance(v, list):
                    o[k] = fix_block(v)
                else:
                    walk(v)
        elif isinstance(o, list):
            for v in o:
                walk(v)

    walk(bir)
    return bir


def patch_bass_serialization(nc):
    orig = nc.to_json_bytes
    nc.to_json_bytes = lambda: json.dumps(
        split_sync_waits(json.loads(orig()))).encode()
    return nc


# ----------------------------------------------------------- device builder

def _clip(nb, dy, dx):
    y0 = nb * 8
    oy0, oy1 = max(y0, -dy), min(y0 + 8, 64 - dy)
    ox0, ox1 = max(0, -dx), min(64, 64 - dx)
    return y0, oy0, oy1, ox0, ox1


def build_nc():
    nc = bass.Bass("TRN2", debug=False, num_devices=8)

    emb_all = nc.dram_tensor("emb_all", [KV, HW], BF16, kind="ExternalInput")
    e_in = {i: nc.dram_tensor(f"e{i}", [CN[i - 1], HW], BF16, kind="ExternalInput")
            for i in range(1, 5)}
    wmkT = nc.dram_tensor("wmkT", [KV, KV], BF16, kind="ExternalInput")
    wmvT = nc.dram_tensor("wmvT", [KV, KV], BF16, kind="ExternalInput")
    dwk = nc.dram_tensor("dwk", [KV, 9, 128], BF16, kind="ExternalInput")
    dwv = nc.dram_tensor("dwv", [KV, 9, 128], BF16, kind="ExternalInput")
    dwkf = nc.dram_tensor("dwkf", [KV, 9], F32, kind="ExternalInput")
    dwvf = nc.dram_tensor("dwvf", [KV, 9], F32, kind="ExternalInput")
    wmT, qb, qwA, qwB, wpT = {}, {}, {}, {}, {}
    for bi, c in enumerate(CN, 1):
        wmT[bi] = nc.dram_tensor(f"wmT{bi}", [c, c], BF16, kind="ExternalInput")
        qb[bi] = nc.dram_tensor(f"qb{bi}", [c, 9, c], BF16, kind="ExternalInput")
        qwA[bi] = nc.dram_tensor(f"qwA{bi}", [c, 9], F32, kind="ExternalInput")
        qwB[bi] = nc.dram_tensor(f"qwB{bi}", [c, 9], F32, kind="ExternalInput")
        wpT[bi] = nc.dram_tensor(f"wpT{bi}", [c, c], BF16, kind="ExternalInput")

    v_sp = nc.dram_tensor("v_sp", [KV, HW], BF16)
    kT_sp = nc.dram_tensor("kT_sp", [NT, 128, KV], BF16)
    out_cat = nc.dram_tensor("out_cat", [KV, HW], F32, kind="ExternalOutput")

    with TileContext(nc) as tc:
        with tc.tile_pool(name="persist", bufs=1) as persist:
            ident = persist.tile([128, 128], BF16, tag="ident")
            masks.make_identity(nc, ident[:])
            ones_bf = persist.tile([128, 1], BF16, tag="ones_bf")
            nc.vector.memset(ones_bf[:], 1.0)

            # ====================== phase 1: k and v ======================
            with (
                tc.tile_pool(name="ph1emb", bufs=1) as ph1emb,
                tc.tile_pool(name="ph1w", bufs=1) as phw,
                tc.tile_pool(name="ph1", bufs=2) as ph,
                tc.tile_pool(name="ps1", bufs=3, space="PSUM") as ps1,
                tc.tile_pool(name="ps2", bufs=3, space="PSUM") as ps2,
                tc.tile_pool(name="pst", bufs=2, space="PSUM") as pst,
            ):
                emb_t = []
                for ti, (o0, p) in enumerate(KV_TILES):
                    t = ph1emb.tile([p, HW], BF16, tag=f"emba{ti}")
                    nc.sync.dma_start(t[:], emb_all[o0:o0 + p, :])
                    emb_t.append(t)
                wt = {}
                for which, wT_d in ((0, wmkT), (1, wmvT)):
                    wl = []
                    for ti, (o0, p) in enumerate(KV_TILES):
                        t = phw.tile([p, KV], BF16, tag=f"w1x1_{which}_{ti}",
                                     name=f"w1x1_{which}_{ti}")
                        nc.scalar.dma_start(t[:], wT_d[o0:o0 + p, :])
                        wl.append(t)
                    wt[which] = wl

                pe_taps = [CENTER_TAP] + [
                    t for t in range(9) if t != CENTER_TAP and t not in DVE_TAPS]
                for mi, (m0, mp) in enumerate(KV_TILES):
                    for which, (dw_d, dwf_d) in enumerate(
                            ((dwk, dwkf), (dwv, dwvf))):
                        is_k = which == 0
                        pfx = "k" if is_k else "v"
                        xpre = ph.tile([mp, HW], BF16, tag=f"{pfx}xpre",
                                       name=f"{pfx}xpre", bufs=1)
                        for nb in range(NB):
                            pt = ps1.tile([mp, 512], F32, tag="pre", name="pre")
                            for kt, (k0, kp) in enumerate(KV_TILES):
                                nc.tensor.matmul(
                                    pt[:], wt[which][kt][:, m0:m0 + mp],
                                    emb_t[kt][:, nb * 512:(nb + 1) * 512],
                                    start=(kt == 0), stop=(kt == 7))
                            nc.scalar.copy(
                                xpre[:, nb * 512:(nb + 1) * 512], pt[:])

                        dwt = ph.tile([mp, 9, 128], BF16, tag=f"{pfx}dwt",
                                      name=f"{pfx}dwt", bufs=1)
                        nc.scalar.dma_start(dwt[:], dw_d[m0:m0 + mp, :, :])
                        dwf = ph.tile([mp, 9], F32, tag=f"{pfx}dwf",
                                      name=f"{pfx}dwf")
                        nc.scalar.dma_start(dwf[:], dwf_d[m0:m0 + mp, :])

                        x3 = xpre[:].rearrange("c (y x) -> c y x", x=64)
                        part = ph.tile([mp, HW], BF16, tag=f"{pfx}part",
                                       name=f"{pfx}part", bufs=1)
                        nc.vector.memset(part[:, 0:64], 0.0)
                        nc.vector.tensor_scalar(
                            part[:, 64:HW], xpre[:, 0:HW - 64],
                            dwf[:, 1:2], None, AT.mult)
                        nc.vector.scalar_tensor_tensor(
                            part[:, 0:HW - 64], xpre[:, 64:HW], dwf[:, 7:8],
                            part[:, 0:HW - 64], AT.mult, AT.add)

                        xdw = ph.tile([mp, HW], BF16, tag=f"{pfx}xdw",
                                      name=f"{pfx}xdw", bufs=1)
                        for nb in range(NB):
                            pt = ps2.tile([mp, 512], F32, tag="dwp", name="dwp")
                            pt3 = pt[:].rearrange("c (y x) -> c y x", x=64)
                            for j, t in enumerate(pe_taps):
                                dy, dx = TAPS[t]
                                y0, oy0, oy1, ox0, ox1 = _clip(nb, dy, dx)
                                if oy1 <= oy0:
                                    continue
                                nc.tensor.matmul(
                                    pt3[:, oy0 - y0:oy1 - y0, ox0:ox1],
                                    dwt[:, t, 0:mp],
                                    x3[:, oy0 + dy:oy1 + dy, ox0 + dx:ox1 + dx],
                                    start=(j == 0),
                                    stop=(j == len(pe_taps) - 1))
                            nc.vector.scalar_tensor_tensor(
                                xdw[:, nb * 512:(nb + 1) * 512], pt[:],
                                1.0, part[:, nb * 512:(nb + 1) * 512],
                                AT.mult, AT.add)

                        if is_k:
                            sq = ph.tile([mp, HW], BF16, tag="sqs", bufs=1)
                            ss = ph.tile([mp, 1], F32, tag="ss")
                            nc.scalar.activation(sq[:], xdw[:], AF.Square,
                                                 accum_out=ss[:])
                            rt = ph.tile([mp, 1], F32, tag="rt")
                            nc.scalar.sqrt(rt[:], ss[:])
                            rs = ph.tile([mp, 1], F32, tag="rs")
                            nc.vector.reciprocal(rs[:], rt[:])
                            xs = ph.tile([mp, HW], BF16, tag="xs")
                            nc.vector.tensor_scalar(xs[:], xdw[:], rs[:],
                                                    None, AT.mult)
                            for cb in range(NT):
                                ptr = pst.tile([128, mp], BF16, tag="tr",
                                               name="tr")
                                nc.tensor.transpose(
                                    ptr[:, 0:mp],
                                    xs[:, cb * 128:(cb + 1) * 128],
                                    ident[0:mp, 0:mp])
                                st = ph.tile([128, mp], BF16, tag="trs",
                                             name="trs")
                                nc.scalar.copy(st[:], ptr[:, 0:mp])
                                nc.scalar.dma_start(
                                    kT_sp[cb, :, m0:m0 + mp], st[:])
                        else:
                            nc.sync.dma_start(v_sp[m0:m0 + mp, :], xdw[:])

            # ====================== phase 2: q branches ======================
            # Q^T is built directly in resident SBUF (no DRAM round-trip).
            # Pool opened here, closed manually after phase 3 (LIFO with ph3).
            qTres_cm = tc.tile_pool(name="qTres", bufs=1)
            qTres_pool = qTres_cm.__enter__()
            qT_res = []
            for n in range(NT):
                qT_res.append(qTres_pool.tile([128, KV], BF16,
                                              tag=f"qTr_{n}",
                                              name=f"qTr_{n}"))
            for bri, bi in enumerate([4, 3, 2, 1]):
                c = CN[bi - 1]
                half = c // 2
                br_off = BR_OFF[bri]
                tiles = _tiles_of(c)
                nti = len(tiles)
                with (
                    tc.tile_pool(name=f"ph2_{bi}", bufs=2) as ph,
                    tc.tile_pool(name="ps3", bufs=3, space="PSUM") as ps3,
                    tc.tile_pool(name="ps4", bufs=3, space="PSUM") as ps4,
                    tc.tile_pool(name="pst2", bufs=2, space="PSUM") as pst,
                ):
                    wmt, et = [], []
                    for kt, (k0, kp) in enumerate(tiles):
                        t = ph.tile([kp, c], BF16, tag=f"wm_{kt}", bufs=1)
                        nc.sync.dma_start(t[:], wmT[bi][k0:k0 + kp, :])
                        wmt.append(t)
                        t2 = ph.tile([kp, HW], BF16, tag=f"e_{kt}", bufs=1)
                        nc.sync.dma_start(t2[:], e_in[bi][k0:k0 + kp, :])
                        et.append(t2)

                    qpre = []
                    for mi, (m0, mp) in enumerate(tiles):
                        xp = ph.tile([mp, HW], BF16, tag=f"qpre_{mi}", bufs=1)
                        for nb in range(NB):
                            pt = ps3.tile([mp, 512], F32, tag="qp", name="qp")
                            for kt, (k0, kp) in enumerate(tiles):
                                nc.tensor.matmul(
                                    pt[:], wmt[kt][:, m0:m0 + mp],
                                    et[kt][:, nb * 512:(nb + 1) * 512],
                                    start=(kt == 0), stop=(kt == nti - 1))
                            nc.scalar.copy(xp[:, nb * 512:(nb + 1) * 512],
                                           pt[:])
                        qpre.append(xp)

                    for mi, (m0, mp) in enumerate(tiles):
                        if c >= 256:
                            a_ti = mi if m0 + mp <= half else mi - nti // 2
                            b_ti = a_ti + nti // 2
                            band_tiles = [a_ti, b_ti]
                            use_dve = True
                        else:
                            band_tiles = [0]
                            use_dve = False

                        part = None
                        if use_dve:
                            qa = ph.tile([mp, 9], F32, tag="qa")
                            qbv = ph.tile([mp, 9], F32, tag="qbv")
                            nc.sync.dma_start(qa[:], qwA[bi][m0:m0 + mp, :])
                            nc.sync.dma_start(qbv[:], qwB[bi][m0:m0 + mp, :])
                            srcA, srcB = qpre[band_tiles[0]], qpre[band_tiles[1]]
                            part = ph.tile([mp, HW], BF16, tag="qpart", bufs=1)
                            nc.vector.memset(part[:, 0:64], 0.0)
                            nc.vector.tensor_scalar(
                                part[:, 64:HW], srcA[:, 0:HW - 64], qa[:, 1:2],
                                None, AT.mult)
                            nc.vector.scalar_tensor_tensor(
                                part[:, 64:HW], srcB[:, 0:HW - 64], qbv[:, 1:2],
                                part[:, 64:HW], AT.mult, AT.add)
                            nc.vector.scalar_tensor_tensor(
                                part[:, 0:HW - 64], srcA[:, 64:HW], qa[:, 7:8],
                                part[:, 0:HW - 64], AT.mult, AT.add)
                            nc.vector.scalar_tensor_tensor(
                                part[:, 0:HW - 64], srcB[:, 64:HW], qbv[:, 7:8],
                                part[:, 0:HW - 64], AT.mult, AT.add)
                            pe_taps = [CENTER_TAP] + [
                                t for t in range(9)
                                if t != CENTER_TAP and t not in DVE_TAPS]
                        else:
                            pe_taps = [CENTER_TAP] + [
                                t for t in range(9) if t != CENTER_TAP]

                        qdw = ph.tile([mp, HW], BF16, tag="qdw")
                        bmats = []
                        for z, ti_src in enumerate(band_tiles):
                            k0b, kpb = tiles[ti_src]
                            bm = ph.tile([kpb, 9, mp], BF16, tag=f"qbm_{z}")
                            nc.sync.dma_start(
                                bm[:], qb[bi][k0b:k0b + kpb, :, m0:m0 + mp])
                            bmats.append((bm, qpre[ti_src]))
                        n_mm = len(pe_taps) * len(bmats)
                        for nb in range(NB):
                            pt = ps4.tile([mp, 512], F32, tag="qdwp",
                                          name="qdwp")
                            pt3 = pt[:].rearrange("c (y x) -> c y x", x=64)
                            j = 0
                            for t in pe_taps:
                                dy, dx = TAPS[t]
                                y0, oy0, oy1, ox0, ox1 = _clip(nb, dy, dx)
                                for (bm, src) in bmats:
                                    if oy1 <= oy0:
                                        j += 1
                                        continue
                                    s3 = src[:].rearrange(
                                        "c (y x) -> c y x", x=64)
                                    nc.tensor.matmul(
                                        pt3[:, oy0 - y0:oy1 - y0, ox0:ox1],
                                        bm[:, t, :],
                                        s3[:, oy0 + dy:oy1 + dy,
                                           ox0 + dx:ox1 + dx],
                                        start=(j == 0), stop=(j == n_mm - 1))
                                    j += 1
                            if use_dve:
                                nc.vector.scalar_tensor_tensor(
                                    qdw[:, nb * 512:(nb + 1) * 512], pt[:],
                                    1.0, part[:, nb * 512:(nb + 1) * 512],
                                    AT.mult, AT.add)
                            else:
                                nc.scalar.copy(
                                    qdw[:, nb * 512:(nb + 1) * 512], pt[:])

                        sq = ph.tile([mp, HW], BF16, tag="qsq", bufs=1)
                        ss = ph.tile([mp, 1], F32, tag="qss")
                        nc.scalar.activation(sq[:], qdw[:], AF.Square,
                                             accum_out=ss[:])
                        rt = ph.tile([mp, 1], F32, tag="qrt")
                        nc.scalar.sqrt(rt[:], ss[:])
                        rs = ph.tile([mp, 1], F32, tag="qrs")
                        nc.vector.reciprocal(rs[:], rt[:])
                        xs = ph.tile([mp, HW], BF16, tag="qxs")
                        nc.vector.tensor_scalar(xs[:], qdw[:], rs[:], None, AT.mult)
                        for cb in range(NT):
                            ptr = pst.tile([128, mp], BF16, tag="qtr",
                                           name="qtr")
                            nc.tensor.transpose(
                                ptr[:, 0:mp], xs[:, cb * 128:(cb + 1) * 128],
                                ident[0:mp, 0:mp])
                            nc.scalar.copy(
                                qT_res[cb][:, br_off + m0:br_off + m0 + mp],
                                ptr[:, 0:mp])

            # ====================== phase 3 ======================
            with (
                tc.tile_pool(name="ph3big", bufs=1) as ph3big,
                tc.tile_pool(name="ph3", bufs=3) as ph3,
            ):
                attn_bf = []
                for mi, (m0, mp) in enumerate(KV_TILES):
                    attn_bf.append(ph3big.tile([mp, KV], BF16, tag=f"attn_{mi}",
                                               name=f"attn_{mi}"))
                for half_i in range(2):
                    ms = list(range(4 * half_i, 4 * half_i + 4))
                    with tc.tile_pool(name="psA", bufs=1, space="PSUM") as psA:
                        pts = {}
                        for mi in ms:
                            m0, mp = KV_TILES[mi]
                            pts[mi] = psA.tile([mp, KV], F32, tag=f"attnp{mi % 4}", name=f"attnp{mi}")
                        for n in range(NT):
                            kTt = ph3.tile([128, KV], BF16, tag="kTt", bufs=4)
                            nc.sync.dma_start(kTt[:], kT_sp[n, :, :])
                            for mi in ms:
                                m0, mp = KV_TILES[mi]
                                for s0, sw in ((0, 512), (512, 448)):
                                    nc.tensor.matmul(
                                        pts[mi][:, s0:s0 + sw],
                                        qT_res[n][:, m0:m0 + mp],
                                        kTt[:, s0:s0 + sw],
                                        start=(n == 0), stop=(n == NT - 1))
                        for mi in ms:
                            nc.scalar.copy(attn_bf[mi][:], pts[mi][:])

                # per-branch stats, vectorized over branches in [1, 4] lanes
                sums = ph3.tile([1, 8], F32, tag="sums")  # cols 0-3 S1, 4-7 S2
                inv_n = ph3.tile([1, 4], F32, tag="inv_n")
                for bri in range(4):
                    nc.vector.memset(inv_n[:, bri:bri + 1],
                                     1.0 / float(BR_C[bri] * KV))
                with (
                    tc.tile_pool(name="psS", bufs=1, space="PSUM") as psS,
                    tc.tile_pool(name="psB", bufs=1, space="PSUM") as psB,
                ):
                    for bri in range(4):
                        mts = BR_MTILES[bri]
                        s1p = psS.tile([1, KV], F32, tag="s1")
                        s2p = psS.tile([1, KV], F32, tag="s2")
                        for j, mi in enumerate(mts):
                            m0, mp = KV_TILES[mi]
                            sqt = ph3.tile([mp, KV], BF16, tag="asq")
                            nc.scalar.activation(sqt[:], attn_bf[mi][:], AF.Square)
                            for s0, sw in ((0, 512), (512, 448)):
                                nc.tensor.matmul(
                                    s1p[:, s0:s0 + sw], ones_bf[0:mp, :],
                                    attn_bf[mi][:, s0:s0 + sw],
                                    start=(j == 0), stop=(j == len(mts) - 1))
                                nc.tensor.matmul(
                                    s2p[:, s0:s0 + sw], ones_bf[0:mp, :],
                                    sqt[:, s0:s0 + sw],
                                    start=(j == 0), stop=(j == len(mts) - 1))
                        s1r = ph3.tile([1, KV], F32, tag="s1r")
                        s2r = ph3.tile([1, KV], F32, tag="s2r")
                        nc.scalar.copy(s1r[:], s1p[:])
                        nc.scalar.copy(s2r[:], s2p[:])
                        nc.vector.tensor_reduce(
                            sums[:, bri:bri + 1], s1r[:], mybir.AxisListType.X,
                            AT.add)
                        nc.vector.tensor_reduce(
                            sums[:, 4 + bri:5 + bri], s2r[:],
                            mybir.AxisListType.X, AT.add)

                    mu = ph3.tile([1, 4], F32, tag="mu")
                    nc.vector.tensor_tensor(mu[:], sums[:, 0:4], inv_n[:], AT.mult)
                    ex2 = ph3.tile([1, 4], F32, tag="ex2")
                    nc.vector.tensor_tensor(ex2[:], sums[:, 4:8], inv_n[:], AT.mult)
                    mu2 = ph3.tile([1, 4], F32, tag="mu2")
                    nc.vector.tensor_tensor(mu2[:], mu[:], mu[:], AT.mult)
                    var = ph3.tile([1, 4], F32, tag="var")
                    nc.vector.tensor_tensor(var[:], ex2[:], mu2[:], AT.subtract)
                    vs = ph3.tile([1, 4], F32, tag="vs")
                    nc.vector.tensor_scalar(vs[:], var[:], SCALE * SCALE,
                                            EPS_IN, AT.mult, AT.add)
                    sd = ph3.tile([1, 4], F32, tag="sd")
                    nc.scalar.sqrt(sd[:], vs[:])
                    rsb = ph3.tile([1, 4], F32, tag="rsb")
                    nc.vector.reciprocal(rsb[:], sd[:])
                    scl = ph3.tile([1, 4], F32, tag="scl")
                    nc.vector.tensor_scalar(scl[:], rsb[:], SCALE, None, AT.mult)
                    bia = ph3.tile([1, 4], F32, tag="bia")
                    nc.vector.tensor_tensor(bia[:], mu[:], scl[:], AT.mult)
                    nc.vector.tensor_scalar(bia[:], bia[:], -1.0, None, AT.mult)
                    onesr_f = ph3.tile([1, 128], F32, tag="onesr_f")
                    nc.vector.memset(onesr_f[:], 1.0)
                    s_ps = psB.tile([128, 4], F32, tag="bps1")
                    b_ps = psB.tile([128, 4], F32, tag="bps2")
                    nc.tensor.matmul(s_ps[:], onesr_f[:], scl[:],
                                     start=True, stop=True)
                    nc.tensor.matmul(b_ps[:], onesr_f[:], bia[:],
                                     start=True, stop=True)
                    sclv4 = ph3.tile([128, 4], F32, tag="sclv4")
                    biav4 = ph3.tile([128, 4], F32, tag="biav4")
                    nc.scalar.copy(sclv4[:], s_ps[:])
                    nc.scalar.copy(biav4[:], b_ps[:])

                # softmax
                probs_bf = []
                for mi, (m0, mp) in enumerate(KV_TILES):
                    bri = BR_OF_M[mi]
                    pb = ph3big.tile([mp, KV], BF16, tag=f"probs_{mi}",
                                     name=f"probs_{mi}")
                    rsum = ph3.tile([mp, 1], F32, tag="rsum")
                    nc.scalar.activation(
                        pb[:], attn_bf[mi][:], AF.Exp,
                        bias=biav4[0:mp, bri:bri + 1],
                        scale=sclv4[0:mp, bri:bri + 1],
                        accum_out=rsum[:])
                    rinv = ph3.tile([mp, 1], F32, tag="rinv")
                    nc.vector.reciprocal(rinv[:], rsum[:])
                    nc.vector.tensor_scalar(pb[:], pb[:], rinv[:], None, AT.mult)
                    probs_bf.append(pb)

                # fold wp into probs: pw[dt][d, co_global]
                pw_bf = []
                for dt, (d0, dp) in enumerate(KV_TILES):
                    pw_bf.append(ph3big.tile([dp, KV], BF16, tag=f"pw_{dt}", name=f"pw_{dt}"))
                with tc.tile_pool(name="psF", bufs=1, space="PSUM") as psF:
                    wpt = {}
                    for bri, bi in enumerate([4, 3, 2, 1]):
                        ts = []
                        for kt, (k0, kp) in enumerate(_tiles_of(BR_C[bri])):
                            t = ph3.tile([kp, BR_C[bri]], BF16,
                                         tag=f"wp_{bri}_{kt}")
                            nc.scalar.dma_start(t[:], wpT[bi][k0:k0 + kp, :])
                            ts.append(t)
                        wpt[bri] = ts
                    for dt, (d0, dp) in enumerate(KV_TILES):
                        pf = psF.tile([dp, KV], F32, tag=f"pf{dt % 4}")
                        for bri in range(4):
                            c, off = BR_C[bri], BR_OFF[bri]
                            tl = _tiles_of(c)
                            for kt, (k0, kp) in enumerate(tl):
                                mi = (off + k0) // 128
                                nc.tensor.matmul(
                                    pf[:, off:off + c],
                                    probs_bf[mi][:, d0:d0 + dp],
                                    wpt[bri][kt][:],
                                    # bank 1 = cols 0:512 (b4); bank 2 =
                                    # cols 512:960 (b3 clears, b2/b1 land on
                                    # unwritten cells -> overwrite)
                                    start=(bri <= 1 and kt == 0),
                                    stop=(kt == len(tl) - 1))
                        nc.scalar.copy(pw_bf[dt][:], pf[:])

                # pv -> final output
                with tc.tile_pool(name="psO", bufs=1, space="PSUM") as psO:
                    for nb in range(NB):
                        vts = []
                        for dt, (d0, dp) in enumerate(KV_TILES):
                            vt = ph3.tile([dp, 512], BF16, tag=f"vt{dt}")
                            nc.scalar.dma_start(
                                vt[:], v_sp[d0:d0 + dp, nb * 512:(nb + 1) * 512])
                            vts.append(vt)
                        for mo, (m0, mp) in enumerate(KV_TILES):
                            po = psO.tile([mp, 512], F32, tag=f"po{mo % 4}")
                            for dt in range(8):
                                nc.tensor.matmul(
                                    po[:], pw_bf[dt][:, m0:m0 + mp], vts[dt][:],
                                    start=(dt == 0), stop=(dt == 7))
                            ot = ph3.tile([mp, 512], F32, tag="ot")
                            nc.scalar.copy(ot[:], po[:])
                            nc.sync.dma_start(
                                out_cat[m0:m0 + mp, nb * 512:(nb + 1) * 512],
                                ot[:])

            qTres_cm.__exit__(None, None, None)

    return nc


# ---------------------------------------------------------------- entry

_CACHE = {}


def _get_nc():
    if "nc" not in _CACHE:
        _CACHE["nc"] = patch_bass_serialization(build_nc())
    return _CACHE["nc"]


def kernel(**inputs):
    ins = {k: np.asarray(v) for k, v in inputs.items()}
    hp = host_prep(ins)
    in_maps = shard_inputs(ins, hp)
    nc = _get_nc()
    res = run_bass_kernel_spmd(nc, in_maps, core_ids=list(range(8)))
    outs = []
    for i in range(4):
        c = CN[i]
        bri = BR_ORDER.index(i)
        r0 = BR_OFF[bri]
        arr = np.stack([
            res.results[b]["out_cat"][r0:r0 + c].reshape(c, 64, 64)
            for b in range(8)
        ])
        outs.append(arr.astype(np.float32))
    return tuple(outs)


# revision 22
# speedup vs baseline: 1.6042x; 1.1287x over previous
"""Trainium2 Bass kernel for nn_Attention_org_1554778161848.

Sharding: data-parallel over batch (B=8 -> 8 NeuronCores), weights replicated.

Per core pipeline (all matmuls bf16 with fp32 PSUM accumulation):
  phase 1: k/v 1x1 convs on PE; depthwise 3x3 as 9 'taps': 7 taps via PE
           diagonal-matmuls accumulated in PSUM (boundary handling via AP
           clipping + PSUM has_written semantics), the 2 aligned taps
           (dy=+-1, dx=0) via DVE scalar_tensor_tensor; l2norm(k) via ACT
           square+accum_out; k^T built with PE transposes -> DRAM; v -> DRAM.
  phase 2: q per branch in concat order [b4,b3,b2,b1] with channels permuted
           even/odd (grouped 3x3 conv becomes 2-band-diagonal); same stencil
           scheme; l2norm; transposes -> Q^T in DRAM.
  phase 3: attn = Q^T.T @ K^T streamed over 32 spatial tiles; per-branch
           InstanceNorm stats via ones-matmuls (literal 1/sqrt(960) scale and
           eps=1e-5 -- eps dominates variance, must match reference exactly);
           softmax = ACT Exp with per-partition scale/bias + accum_out row
           sums; wp is folded into probs ((wp@p)@v == wp@(p@v)); pv emits the
           final output.
"""
import json
import math

import numpy as np
import ml_dtypes

import concourse.bass as bass
import concourse.mybir as mybir
from concourse import masks
from concourse.tile import TileContext
from concourse.bass_utils import run_bass_kernel_spmd

BF16 = mybir.dt.bfloat16
F32 = mybir.dt.float32
AT = mybir.AluOpType
AF = mybir.ActivationFunctionType

CN = [64, 128, 256, 512]
KV = 960
HW = 4096
SCALE = 1.0 / math.sqrt(KV)
EPS_IN = 1e-5
BR_ORDER = [3, 2, 1, 0]
BR_C = [512, 256, 128, 64]
BR_OFF = [0, 512, 768, 896]
TAPS = [(dy, dx) for dy in (-1, 0, 1) for dx in (-1, 0, 1)]  # row-major, matches wk/wq reshape
DVE_TAPS = (1, 7)  # (-1,0), (+1,0): 64-element shifts, bf16-aligned for DVE
CENTER_TAP = 4     # (0,0): full coverage, must run first so start=True clears the bank
KV_TILES = [(0, 128), (128, 128), (256, 128), (384, 128),
            (512, 128), (640, 128), (768, 128), (896, 64)]
NB = 8
NT = 32
BR_MTILES = [[0, 1, 2, 3], [4, 5], [6], [7]]
BR_OF_M = [0, 0, 0, 0, 1, 1, 2, 3]


def _tiles_of(c):
    return [(0, 64)] if c == 64 else [(i * 128, 128) for i in range(c // 128)]


# ---------------------------------------------------------------- host prep

def perm_for(c):
    """new index j -> old channel; per 128-tile: [64 evens | 64 odds]."""
    tile = min(c, 128)
    h = tile // 2
    old = np.empty(c, np.int64)
    for t0 in range(0, c, tile):
        g0 = t0 // 2
        old[t0:t0 + h] = 2 * (g0 + np.arange(h))
        old[t0 + h:t0 + tile] = 2 * (g0 + np.arange(h)) + 1
    return old


def _bf(x):
    return np.ascontiguousarray(np.asarray(x, np.float32)).astype(ml_dtypes.bfloat16)


def host_prep(ins):
    d = {}
    d["wmkT"] = _bf(np.asarray(ins["wmk"])[:, :, 0, 0].T)
    d["wmvT"] = _bf(np.asarray(ins["wmv"])[:, :, 0, 0].T)
    for nm, w in (("dwk", ins["wk"]), ("dwv", ins["wv"])):
        w9 = np.asarray(w)[:, 0].reshape(KV, 9).astype(np.float32)
        blk = np.zeros((KV, 9, 128), np.float32)
        for o0, p in KV_TILES:
            for i in range(p):
                blk[o0 + i, :, i] = w9[o0 + i]
        d[nm] = _bf(blk)
        d[nm + "f"] = np.ascontiguousarray(w9)  # fp32 per-channel tap weights
    for bi, c in enumerate(CN, 1):
        old = perm_for(c)
        half = c // 2
        wm = np.asarray(ins[f"wm{bi}"])[:, :, 0, 0]
        d[f"wmT{bi}"] = _bf(wm.T[:, old])
        wq = np.asarray(ins[f"wq{bi}"]).reshape(c, 2, 9).astype(np.float32)
        G = np.zeros((c, 9, c), np.float32)
        inv = np.empty(c, np.int64)
        inv[old] = np.arange(c)
        qwS = np.zeros((c, 9), np.float32)
        for j in range(c):
            o = old[j]
            g = o // 2
            G[inv[2 * g], :, j] = wq[o, 0, :]
            G[inv[2 * g + 1], :, j] = wq[o, 1, :]
            qwS[j] = wq[o, 0 if o % 2 == 0 else 1, :]
        for t in DVE_TAPS:  # self-contribution of dy taps handled on DVE
            for j in range(c):
                G[j, t, j] = 0.0
        d[f"qb{bi}"] = _bf(G)
        d[f"qwS{bi}"] = qwS
        wp = np.asarray(ins[f"wp{bi}"])[:, :, 0, 0]
        d[f"wpT{bi}"] = _bf(wp.T[old, :])
    return d


def shard_inputs(ins, hp):
    emb_all = np.asarray(ins["emb_all"]).reshape(8, KV, HW)
    embs = [np.asarray(ins[f"emb{i}"]).reshape(8, CN[i - 1], HW) for i in range(1, 5)]
    maps = []
    for b in range(8):
        m = {"emb_all": _bf(emb_all[b])}
        for i in range(1, 5):
            m[f"e{i}"] = _bf(embs[i - 1][b])
        m.update(hp)
        maps.append(m)
    return maps


# ------------------------------------------------- walrus 1-wait workaround

def split_sync_waits(bir, limit=1):
    def fix_block(instrs):
        out = []
        for ins in instrs:
            si = ins.get("sync_info") or {}
            waits = si.get("on_wait") or []
            if len(waits) > limit:
                chunks = [waits[i:i + limit] for i in range(0, len(waits), limit)]
                for j, ch in enumerate(chunks[:-1]):
                    out.append({
                        "name": ins["name"] + f"-w{j}", "opcode": "Drain",
                        "engine": ins["engine"], "ins": [], "outs": [],
                        "is_reset_sema": False,
                        "sync_info": {"on_update": [], "on_wait": ch},
                        "debug": ins.get("debug"),
                    })
                ins["sync_info"]["on_wait"] = chunks[-1]
            out.append(ins)
        return out

    def walk(o):
        if isinstance(o, dict):
            for k, v in o.items():
                if k == "instructions" and isinstance(v, list):
                    o[k] = fix_block(v)
                else:
                    walk(v)
        elif isinstance(o, list):
            for v in o:
                walk(v)

    walk(bir)
    return bir


def patch_bass_serialization(nc):
    orig = nc.to_json_bytes
    nc.to_json_bytes = lambda: json.dumps(
        split_sync_waits(json.loads(orig()))).encode()
    return nc


# ----------------------------------------------------------- device builder

def _clip(nb, dy, dx):
    y0 = nb * 8
    oy0, oy1 = max(y0, -dy), min(y0 + 8, 64 - dy)
    ox0, ox1 = max(0, -dx), min(64, 64 - dx)
    return y0, oy0, oy1, ox0, ox1


def build_nc():
    nc = bass.Bass("TRN2", debug=False, num_devices=8)

    emb_all = nc.dram_tensor("emb_all", [KV, HW], BF16, kind="ExternalInput")
    e_in = {i: nc.dram_tensor(f"e{i}", [CN[i - 1], HW], BF16, kind="ExternalInput")
            for i in range(1, 5)}
    wmkT = nc.dram_tensor("wmkT", [KV, KV], BF16, kind="ExternalInput")
    wmvT = nc.dram_tensor("wmvT", [KV, KV], BF16, kind="ExternalInput")
    dwk = nc.dram_tensor("dwk", [KV, 9, 128], BF16, kind="ExternalInput")
    dwv = nc.dram_tensor("dwv", [KV, 9, 128], BF16, kind="ExternalInput")
    dwkf = nc.dram_tensor("dwkf", [KV, 9], F32, kind="ExternalInput")
    dwvf = nc.dram_tensor("dwvf", [KV, 9], F32, kind="ExternalInput")
    wmT, qb, qwS, wpT = {}, {}, {}, {}
    for bi, c in enumerate(CN, 1):
        wmT[bi] = nc.dram_tensor(f"wmT{bi}", [c, c], BF16, kind="ExternalInput")
        qb[bi] = nc.dram_tensor(f"qb{bi}", [c, 9, c], BF16, kind="ExternalInput")
        qwS[bi] = nc.dram_tensor(f"qwS{bi}", [c, 9], F32, kind="ExternalInput")
        wpT[bi] = nc.dram_tensor(f"wpT{bi}", [c, c], BF16, kind="ExternalInput")

    v_sp = nc.dram_tensor("v_sp", [KV, HW], BF16)
    kT_sp = nc.dram_tensor("kT_sp", [NT, 128, KV], BF16)
    out_cat = nc.dram_tensor("out_cat", [KV, HW], F32, kind="ExternalOutput")

    with TileContext(nc) as tc:
        with tc.tile_pool(name="persist", bufs=1) as persist:
            ident = persist.tile([128, 128], BF16, tag="ident")
            masks.make_identity(nc, ident[:])
            ones_bf = persist.tile([128, 1], BF16, tag="ones_bf")
            nc.vector.memset(ones_bf[:], 1.0)

            # ====================== phase 1: k and v ======================
            with (
                tc.tile_pool(name="ph1emb", bufs=1) as ph1emb,
                tc.tile_pool(name="ph1w", bufs=1) as phw,
                tc.tile_pool(name="ph1", bufs=2) as ph,
                tc.tile_pool(name="ps1", bufs=3, space="PSUM") as ps1,
                tc.tile_pool(name="ps2", bufs=3, space="PSUM") as ps2,
                tc.tile_pool(name="pst", bufs=2, space="PSUM") as pst,
            ):
                emb_t = []
                for ti, (o0, p) in enumerate(KV_TILES):
                    t = ph1emb.tile([p, HW], BF16, tag=f"emba{ti}")
                    nc.sync.dma_start(t[:], emb_all[o0:o0 + p, :])
                    emb_t.append(t)
                wt = {}
                for which, wT_d in ((0, wmkT), (1, wmvT)):
                    wl = []
                    for ti, (o0, p) in enumerate(KV_TILES):
                        t = phw.tile([p, KV], BF16, tag=f"w1x1_{which}_{ti}",
                                     name=f"w1x1_{which}_{ti}")
                        nc.scalar.dma_start(t[:], wT_d[o0:o0 + p, :])
                        wl.append(t)
                    wt[which] = wl

                pe_taps = [CENTER_TAP] + [
                    t for t in range(9) if t != CENTER_TAP and t not in DVE_TAPS]
                for mi, (m0, mp) in enumerate(KV_TILES):
                    for which, (dw_d, dwf_d) in enumerate(
                            ((dwk, dwkf), (dwv, dwvf))):
                        is_k = which == 0
                        pfx = "k" if is_k else "v"
                        xpre = ph.tile([mp, HW], BF16, tag=f"{pfx}xpre",
                                       name=f"{pfx}xpre", bufs=1)
                        for nb in range(NB):
                            pt = ps1.tile([mp, 512], F32, tag="pre", name="pre")
                            for kt, (k0, kp) in enumerate(KV_TILES):
                                nc.tensor.matmul(
                                    pt[:], wt[which][kt][:, m0:m0 + mp],
                                    emb_t[kt][:, nb * 512:(nb + 1) * 512],
                                    start=(kt == 0), stop=(kt == 7))
                            nc.scalar.copy(
                                xpre[:, nb * 512:(nb + 1) * 512], pt[:])

                        dwt = ph.tile([mp, 9, 128], BF16, tag=f"{pfx}dwt",
                                      name=f"{pfx}dwt", bufs=1)
                        nc.scalar.dma_start(dwt[:], dw_d[m0:m0 + mp, :, :])
                        dwf = ph.tile([mp, 9], F32, tag=f"{pfx}dwf",
                                      name=f"{pfx}dwf")
                        nc.scalar.dma_start(dwf[:], dwf_d[m0:m0 + mp, :])

                        x3 = xpre[:].rearrange("c (y x) -> c y x", x=64)
                        part = ph.tile([mp, HW], BF16, tag=f"{pfx}part",
                                       name=f"{pfx}part", bufs=1)
                        nc.vector.memset(part[:, 0:64], 0.0)
                        nc.vector.tensor_scalar(
                            part[:, 64:HW], xpre[:, 0:HW - 64],
                            dwf[:, 1:2], None, AT.mult)
                        nc.vector.scalar_tensor_tensor(
                            part[:, 0:HW - 64], xpre[:, 64:HW], dwf[:, 7:8],
                            part[:, 0:HW - 64], AT.mult, AT.add)

                        xdw = ph.tile([mp, HW], BF16, tag=f"{pfx}xdw",
                                      name=f"{pfx}xdw", bufs=1)
                        for nb in range(NB):
                            pt = ps2.tile([mp, 512], F32, tag="dwp", name="dwp")
                            pt3 = pt[:].rearrange("c (y x) -> c y x", x=64)
                            for j, t in enumerate(pe_taps):
                                dy, dx = TAPS[t]
                                y0, oy0, oy1, ox0, ox1 = _clip(nb, dy, dx)
                                if oy1 <= oy0:
                                    continue
                                nc.tensor.matmul(
                                    pt3[:, oy0 - y0:oy1 - y0, ox0:ox1],
                                    dwt[:, t, 0:mp],
                                    x3[:, oy0 + dy:oy1 + dy, ox0 + dx:ox1 + dx],
                                    start=(j == 0),
                                    stop=(j == len(pe_taps) - 1))
                            nc.vector.scalar_tensor_tensor(
                                xdw[:, nb * 512:(nb + 1) * 512], pt[:],
                                1.0, part[:, nb * 512:(nb + 1) * 512],
                                AT.mult, AT.add)

                        if is_k:
                            sq = ph.tile([mp, HW], BF16, tag="sqs", bufs=1)
                            ss = ph.tile([mp, 1], F32, tag="ss")
                            nc.scalar.activation(sq[:], xdw[:], AF.Square,
                                                 accum_out=ss[:])
                            rt = ph.tile([mp, 1], F32, tag="rt")
                            nc.scalar.sqrt(rt[:], ss[:])
                            rs = ph.tile([mp, 1], F32, tag="rs")
                            nc.vector.reciprocal(rs[:], rt[:])
                            xs = ph.tile([mp, HW], BF16, tag="xs")
                            nc.vector.tensor_scalar(xs[:], xdw[:], rs[:],
                                                    None, AT.mult)
                            for cb in range(NT):
                                ptr = pst.tile([128, mp], BF16, tag="tr",
                                               name="tr")
                                nc.tensor.transpose(
                                    ptr[:, 0:mp],
                                    xs[:, cb * 128:(cb + 1) * 128],
                                    ident[0:mp, 0:mp])
                                st = ph.tile([128, mp], BF16, tag="trs",
                                             name="trs")
                                nc.scalar.copy(st[:], ptr[:, 0:mp])
                                nc.scalar.dma_start(
                                    kT_sp[cb, :, m0:m0 + mp], st[:])
                        else:
                            nc.sync.dma_start(v_sp[m0:m0 + mp, :], xdw[:])

            # ====================== phase 2: q branches ======================
            # Q^T is built directly in resident SBUF (no DRAM round-trip).
            # Pool opened here, closed manually after phase 3 (LIFO with ph3).
            qTres_cm = tc.tile_pool(name="qTres", bufs=1)
            qTres_pool = qTres_cm.__enter__()
            qT4_res, qTr_res = [], []
            for n in range(NT):
                qT4_res.append(qTres_pool.tile([128, 512], BF16,
                                               tag=f"qT4_{n}",
                                               name=f"qT4_{n}"))
                qTr_res.append(qTres_pool.tile([128, KV - 512], BF16,
                                               tag=f"qTr_{n}",
                                               name=f"qTr_{n}"))
            for bri, bi in enumerate([4, 3, 2, 1]):
                c = CN[bi - 1]
                half = c // 2
                br_off = BR_OFF[bri]
                tiles = _tiles_of(c)
                nti = len(tiles)
                with (
                    tc.tile_pool(name=f"ph2_{bi}", bufs=2) as ph,
                    tc.tile_pool(name="ps3", bufs=3, space="PSUM") as ps3,
                    tc.tile_pool(name="ps4", bufs=3, space="PSUM") as ps4,
                    tc.tile_pool(name="pst2", bufs=2, space="PSUM") as pst,
                ):
                    wmt, et = [], []
                    for kt, (k0, kp) in enumerate(tiles):
                        t = ph.tile([kp, c], BF16, tag=f"wm_{kt}", bufs=1)
                        nc.sync.dma_start(t[:], wmT[bi][k0:k0 + kp, :])
                        wmt.append(t)
                        t2 = ph.tile([kp, HW], BF16, tag=f"e_{kt}", bufs=1)
                        nc.sync.dma_start(t2[:], e_in[bi][k0:k0 + kp, :])
                        et.append(t2)

                    qpre = []
                    for mi, (m0, mp) in enumerate(tiles):
                        xp = ph.tile([mp, HW], BF16, tag=f"qpre_{mi}", bufs=1)
                        for nb in range(NB):
                            pt = ps3.tile([mp, 512], F32, tag="qp", name="qp")
                            for kt, (k0, kp) in enumerate(tiles):
                                nc.tensor.matmul(
                                    pt[:], wmt[kt][:, m0:m0 + mp],
                                    et[kt][:, nb * 512:(nb + 1) * 512],
                                    start=(kt == 0), stop=(kt == nti - 1))
                            nc.scalar.copy(xp[:, nb * 512:(nb + 1) * 512],
                                           pt[:])
                        qpre.append(xp)

                    for mi, (m0, mp) in enumerate(tiles):
                        # tile-local perm => band matrix is block (mi, mi);
                        # self-contribution of the dy taps runs on DVE.
                        qa = ph.tile([mp, 9], F32, tag="qa")
                        nc.sync.dma_start(qa[:], qwS[bi][m0:m0 + mp, :])
                        srcS = qpre[mi]
                        part = ph.tile([mp, HW], BF16, tag="qpart", bufs=1)
                        nc.vector.memset(part[:, 0:64], 0.0)
                        nc.vector.tensor_scalar(
                            part[:, 64:HW], srcS[:, 0:HW - 64], qa[:, 1:2],
                            None, AT.mult)
                        nc.vector.scalar_tensor_tensor(
                            part[:, 0:HW - 64], srcS[:, 64:HW], qa[:, 7:8],
                            part[:, 0:HW - 64], AT.mult, AT.add)
                        use_dve = True
                        pe_taps = [CENTER_TAP] + [
                            t for t in range(9) if t != CENTER_TAP]

                        qdw = ph.tile([mp, HW], BF16, tag="qdw")
                        bm = ph.tile([mp, 9, mp], BF16, tag="qbm_0")
                        nc.sync.dma_start(
                            bm[:], qb[bi][m0:m0 + mp, :, m0:m0 + mp])
                        bmats = [(bm, srcS)]
                        n_mm = len(pe_taps) * len(bmats)
                        for nb in range(NB):
                            pt = ps4.tile([mp, 512], F32, tag="qdwp",
                                          name="qdwp")
                            pt3 = pt[:].rearrange("c (y x) -> c y x", x=64)
                            j = 0
                            for t in pe_taps:
                                dy, dx = TAPS[t]
                                y0, oy0, oy1, ox0, ox1 = _clip(nb, dy, dx)
                                for (bm, src) in bmats:
                                    if oy1 <= oy0:
                                        j += 1
                                        continue
                                    s3 = src[:].rearrange(
                                        "c (y x) -> c y x", x=64)
                                    nc.tensor.matmul(
                                        pt3[:, oy0 - y0:oy1 - y0, ox0:ox1],
                                        bm[:, t, :],
                                        s3[:, oy0 + dy:oy1 + dy,
                                           ox0 + dx:ox1 + dx],
                                        start=(j == 0), stop=(j == n_mm - 1))
                                    j += 1
                            if use_dve:
                                nc.vector.scalar_tensor_tensor(
                                    qdw[:, nb * 512:(nb + 1) * 512], pt[:],
                                    1.0, part[:, nb * 512:(nb + 1) * 512],
                                    AT.mult, AT.add)
                            else:
                                nc.scalar.copy(
                                    qdw[:, nb * 512:(nb + 1) * 512], pt[:])

                        sq = ph.tile([mp, HW], BF16, tag="qsq", bufs=1)
                        ss = ph.tile([mp, 1], F32, tag="qss")
                        nc.scalar.activation(sq[:], qdw[:], AF.Square,
                                             accum_out=ss[:])
                        rt = ph.tile([mp, 1], F32, tag="qrt")
                        nc.scalar.sqrt(rt[:], ss[:])
                        rs = ph.tile([mp, 1], F32, tag="qrs")
                        nc.vector.reciprocal(rs[:], rt[:])
                        xs = ph.tile([mp, HW], BF16, tag="qxs")
                        nc.vector.tensor_scalar(xs[:], qdw[:], rs[:], None, AT.mult)
                        for cb in range(NT):
                            ptr = pst.tile([128, mp], BF16, tag="qtr",
                                           name="qtr")
                            nc.tensor.transpose(
                                ptr[:, 0:mp], xs[:, cb * 128:(cb + 1) * 128],
                                ident[0:mp, 0:mp])
                            if bi == 4:
                                nc.scalar.copy(
                                    qT4_res[cb][:, m0:m0 + mp], ptr[:, 0:mp])
                            else:
                                o2 = br_off + m0 - 512
                                nc.scalar.copy(
                                    qTr_res[cb][:, o2:o2 + mp], ptr[:, 0:mp])

            # ====================== phase 3 ======================
            with (
                tc.tile_pool(name="ph3big", bufs=1) as ph3big,
                tc.tile_pool(name="ph3", bufs=3) as ph3,
            ):
                attn_bf = []
                for mi, (m0, mp) in enumerate(KV_TILES):
                    attn_bf.append(ph3big.tile([mp, KV], BF16, tag=f"attn_{mi}",
                                               name=f"attn_{mi}"))
                for half_i in range(2):
                    ms = list(range(4 * half_i, 4 * half_i + 4))
                    with tc.tile_pool(name="psA", bufs=1, space="PSUM") as psA:
                        pts = {}
                        for mi in ms:
                            m0, mp = KV_TILES[mi]
                            pts[mi] = psA.tile([mp, KV], F32, tag=f"attnp{mi % 4}", name=f"attnp{mi}")
                        for n in range(NT):
                            kTt = ph3.tile([128, KV], BF16, tag="kTt", bufs=4)
                            nc.sync.dma_start(kTt[:], kT_sp[n, :, :])
                            for mi in ms:
                                m0, mp = KV_TILES[mi]
                                for s0, sw in ((0, 512), (512, 448)):
                                    qsrc = (qT4_res[n][:, m0:m0 + mp]
                                            if mi < 4 else
                                            qTr_res[n][:, m0 - 512:m0 - 512 + mp])
                                    nc.tensor.matmul(
                                        pts[mi][:, s0:s0 + sw],
                                        qsrc,
                                        kTt[:, s0:s0 + sw],
                                        start=(n == 0), stop=(n == NT - 1))
                        for mi in ms:
                            nc.scalar.copy(attn_bf[mi][:], pts[mi][:])

                # per-branch stats, vectorized over branches in [1, 4] lanes
                sums = ph3.tile([1, 8], F32, tag="sums")  # cols 0-3 S1, 4-7 S2
                inv_n = ph3.tile([1, 4], F32, tag="inv_n")
                for bri in range(4):
                    nc.vector.memset(inv_n[:, bri:bri + 1],
                                     1.0 / float(BR_C[bri] * KV))
                with (
                    tc.tile_pool(name="psS", bufs=1, space="PSUM") as psS,
                    tc.tile_pool(name="psB", bufs=1, space="PSUM") as psB,
                ):
                    for bri in range(4):
                        mts = BR_MTILES[bri]
                        s1p = psS.tile([1, KV], F32, tag="s1")
                        s2p = psS.tile([1, KV], F32, tag="s2")
                        for j, mi in enumerate(mts):
                            m0, mp = KV_TILES[mi]
                            sqt = ph3.tile([mp, KV], BF16, tag="asq")
                            nc.scalar.activation(sqt[:], attn_bf[mi][:], AF.Square)
                            for s0, sw in ((0, 512), (512, 448)):
                                nc.tensor.matmul(
                                    s1p[:, s0:s0 + sw], ones_bf[0:mp, :],
                                    attn_bf[mi][:, s0:s0 + sw],
                                    start=(j == 0), stop=(j == len(mts) - 1))
                                nc.tensor.matmul(
                                    s2p[:, s0:s0 + sw], ones_bf[0:mp, :],
                                    sqt[:, s0:s0 + sw],
                                    start=(j == 0), stop=(j == len(mts) - 1))
                        s1r = ph3.tile([1, KV], F32, tag="s1r")
                        s2r = ph3.tile([1, KV], F32, tag="s2r")
                        nc.scalar.copy(s1r[:], s1p[:])
                        nc.scalar.copy(s2r[:], s2p[:])
                        nc.vector.tensor_reduce(
                            sums[:, bri:bri + 1], s1r[:], mybir.AxisListType.X,
                            AT.add)
                        nc.vector.tensor_reduce(
                            sums[:, 4 + bri:5 + bri], s2r[:],
                            mybir.AxisListType.X, AT.add)

                    mu = ph3.tile([1, 4], F32, tag="mu")
                    nc.vector.tensor_tensor(mu[:], sums[:, 0:4], inv_n[:], AT.mult)
                    ex2 = ph3.tile([1, 4], F32, tag="ex2")
                    nc.vector.tensor_tensor(ex2[:], sums[:, 4:8], inv_n[:], AT.mult)
                    mu2 = ph3.tile([1, 4], F32, tag="mu2")
                    nc.vector.tensor_tensor(mu2[:], mu[:], mu[:], AT.mult)
                    var = ph3.tile([1, 4], F32, tag="var")
                    nc.vector.tensor_tensor(var[:], ex2[:], mu2[:], AT.subtract)
                    vs = ph3.tile([1, 4], F32, tag="vs")
                    nc.vector.tensor_scalar(vs[:], var[:], SCALE * SCALE,
                                            EPS_IN, AT.mult, AT.add)
                    sd = ph3.tile([1, 4], F32, tag="sd")
                    nc.scalar.sqrt(sd[:], vs[:])
                    rsb = ph3.tile([1, 4], F32, tag="rsb")
                    nc.vector.reciprocal(rsb[:], sd[:])
                    scl = ph3.tile([1, 4], F32, tag="scl")
                    nc.vector.tensor_scalar(scl[:], rsb[:], SCALE, None, AT.mult)
                    bia = ph3.tile([1, 4], F32, tag="bia")
                    nc.vector.tensor_tensor(bia[:], mu[:], scl[:], AT.mult)
                    nc.vector.tensor_scalar(bia[:], bia[:], -1.0, None, AT.mult)
                    onesr_f = ph3.tile([1, 128], F32, tag="onesr_f")
                    nc.vector.memset(onesr_f[:], 1.0)
                    s_ps = psB.tile([128, 4], F32, tag="bps1")
                    b_ps = psB.tile([128, 4], F32, tag="bps2")
                    nc.tensor.matmul(s_ps[:], onesr_f[:], scl[:],
                                     start=True, stop=True)
                    nc.tensor.matmul(b_ps[:], onesr_f[:], bia[:],
                                     start=True, stop=True)
                    sclv4 = ph3.tile([128, 4], F32, tag="sclv4")
                    biav4 = ph3.tile([128, 4], F32, tag="biav4")
                    nc.scalar.copy(sclv4[:], s_ps[:])
                    nc.scalar.copy(biav4[:], b_ps[:])

                # softmax
                probs_bf = []
                for mi, (m0, mp) in enumerate(KV_TILES):
                    bri = BR_OF_M[mi]
                    pb = ph3big.tile([mp, KV], BF16, tag=f"probs_{mi}",
                                     name=f"probs_{mi}")
                    rsum = ph3.tile([mp, 1], F32, tag="rsum")
                    nc.scalar.activation(
                        pb[:], attn_bf[mi][:], AF.Exp,
                        bias=biav4[0:mp, bri:bri + 1],
                        scale=sclv4[0:mp, bri:bri + 1],
                        accum_out=rsum[:])
                    rinv = ph3.tile([mp, 1], F32, tag="rinv")
                    nc.vector.reciprocal(rinv[:], rsum[:])
                    nc.vector.tensor_scalar(pb[:], pb[:], rinv[:], None, AT.mult)
                    probs_bf.append(pb)

                # fold wp into probs: pw[dt][d, co_global]
                pw_bf = []
                for dt, (d0, dp) in enumerate(KV_TILES):
                    pw_bf.append(ph3big.tile([dp, KV], BF16, tag=f"pw_{dt}", name=f"pw_{dt}"))
                with tc.tile_pool(name="psF", bufs=1, space="PSUM") as psF:
                    wpt = {}
                    for bri, bi in enumerate([4, 3, 2, 1]):
                        ts = []
                        for kt, (k0, kp) in enumerate(_tiles_of(BR_C[bri])):
                            t = ph3.tile([kp, BR_C[bri]], BF16,
                                         tag=f"wp_{bri}_{kt}")
                            nc.scalar.dma_start(t[:], wpT[bi][k0:k0 + kp, :])
                            ts.append(t)
                        wpt[bri] = ts
                    for dt, (d0, dp) in enumerate(KV_TILES):
                        pf = psF.tile([dp, KV], F32, tag=f"pf{dt % 4}")
                        for bri in range(4):
                            c, off = BR_C[bri], BR_OFF[bri]
                            tl = _tiles_of(c)
                            for kt, (k0, kp) in enumerate(tl):
                                mi = (off + k0) // 128
                                nc.tensor.matmul(
                                    pf[:, off:off + c],
                                    probs_bf[mi][:, d0:d0 + dp],
                                    wpt[bri][kt][:],
                                    # bank 1 = cols 0:512 (b4); bank 2 =
                                    # cols 512:960 (b3 clears, b2/b1 land on
                                    # unwritten cells -> overwrite)
                                    start=(bri <= 1 and kt == 0),
                                    stop=(kt == len(tl) - 1))
                        nc.scalar.copy(pw_bf[dt][:], pf[:])

                # pv -> final output
                with tc.tile_pool(name="psO", bufs=1, space="PSUM") as psO:
                    for nb in range(NB):
                        vts = []
                        for dt, (d0, dp) in enumerate(KV_TILES):
                            vt = ph3.tile([dp, 512], BF16, tag=f"vt{dt}")
                            nc.scalar.dma_start(
                                vt[:], v_sp[d0:d0 + dp, nb * 512:(nb + 1) * 512])
                            vts.append(vt)
                        for mo, (m0, mp) in enumerate(KV_TILES):
                            po = psO.tile([mp, 512], F32, tag=f"po{mo % 4}")
                            for dt in range(8):
                                nc.tensor.matmul(
                                    po[:], pw_bf[dt][:, m0:m0 + mp], vts[dt][:],
                                    start=(dt == 0), stop=(dt == 7))
                            ot = ph3.tile([mp, 512], F32, tag="ot")
                            nc.scalar.copy(ot[:], po[:])
                            nc.sync.dma_start(
                                out_cat[m0:m0 + mp, nb * 512:(nb + 1) * 512],
                                ot[:])

            qTres_cm.__exit__(None, None, None)

    return nc


# ---------------------------------------------------------------- entry

_CACHE = {}


def _get_nc():
    if "nc" not in _CACHE:
        _CACHE["nc"] = patch_bass_serialization(build_nc())
    return _CACHE["nc"]


def kernel(**inputs):
    ins = {k: np.asarray(v) for k, v in inputs.items()}
    hp = host_prep(ins)
    in_maps = shard_inputs(ins, hp)
    nc = _get_nc()
    res = run_bass_kernel_spmd(nc, in_maps, core_ids=list(range(8)))
    outs = []
    for i in range(4):
        c = CN[i]
        bri = BR_ORDER.index(i)
        r0 = BR_OFF[bri]
        arr = np.stack([
            res.results[b]["out_cat"][r0:r0 + c].reshape(c, 64, 64)
            for b in range(8)
        ])
        outs.append(arr.astype(np.float32))
    return tuple(outs)
